# revision 9
# baseline (speedup 1.0000x reference)
"""Trainium2 Bass kernel for nn_DirectionalDiagram.

out[f, i, j] = X[f, i] + Y[f, j] + x[i, j]        f in [64], i,j in [1024]
  X[f, i] = (cos(t_f) - idx[i]) * 0.5 * cos(t_f)
  Y[f, j] = (sin(t_f) - idx[j]) * 0.5 * sin(t_f)
  idx[i]  = (i - 511.5) / (1024 * sqrt(2))

Sharding: the filter axis is split across the 8 NeuronCores (8 filters per
core); x is replicated.  Per core the kernel is output-bandwidth bound
(32 MiB of f32 writes); the whole computation is a single fused
scalar_tensor_tensor per [128, 1024] output tile:
    out_tile = (x_tile + X_col[f,b]) + YB[f]
where X_col is a per-partition scalar column and YB[f] is Y[f, :]
broadcast across partitions (built once with gpsimd partition_broadcast).
"""

import numpy as np

W = 1024          # image side
P = 128           # SBUF partitions
NB = W // P       # 8 row-blocks
F_TOTAL = 64
N_CORES = 8
F_LOC = F_TOTAL // N_CORES   # 8 filters per core
GH = 4            # row-blocks per output DMA (2 MiB per dma_start)

TRACE = False     # set by test harness to capture an NTFF profile
LAST_RESULT = None

_module_cache = {}


def _build_module():
    import concourse.bacc as bacc
    import concourse.mybir as mybir
    from concourse import tile

    fp32 = mybir.dt.float32
    AOP = mybir.AluOpType

    nc = bacc.Bacc("TRN2", target_bir_lowering=False, debug=False)
    x_d = nc.dram_tensor("x", [W, W], fp32, kind="ExternalInput").ap()
    cs_d = nc.dram_tensor("cs", [2, F_LOC], fp32, kind="ExternalInput").ap()
    idx8_d = nc.dram_tensor("idx8", [F_LOC, W], fp32, kind="ExternalInput").ap()
    idxcol_d = nc.dram_tensor("idxcol", [P, NB], fp32, kind="ExternalInput").ap()
    out_d = nc.dram_tensor("out", [F_LOC, W, W], fp32, kind="ExternalOutput").ap()

    with tile.TileContext(nc) as tc:
        with (
            tc.tile_pool(name="const", bufs=1) as cpool,
            tc.tile_pool(name="outp", bufs=4) as opool,
            tc.tile_pool(name="dscratch", bufs=1, space="DRAM") as dpool,
        ):
            # ---- tiny inputs first (sync ring) so the setup chain is not
            # queued behind the 4 MiB x load (scalar ring) ----
            c_row = cpool.tile([1, F_LOC], fp32)
            nc.sync.dma_start(out=c_row[:, :], in_=cs_d[0:1, :])
            s_col = cpool.tile([F_LOC, 1], fp32)
            nc.sync.dma_start(out=s_col[:, :], in_=cs_d[1:2, :].transpose([1, 0]))
            idx8_sb = cpool.tile([F_LOC, W], fp32)
            nc.sync.dma_start(out=idx8_sb[:, :], in_=idx8_d[:, :])
            idxcol_sb = cpool.tile([P, NB], fp32)
            nc.sync.dma_start(out=idxcol_sb[:, :], in_=idxcol_d[:, :])

            # ---- load x: [1024,1024] -> [128, 8*1024] (block b at cols b*W),
            # in two halves on the scalar HWDGE ring ----
            x_sb = cpool.tile([P, NB * W], fp32)
            for half in range(2):
                hb = NB // 2
                nc.scalar.dma_start(
                    out=x_sb[:, half * hb * W : (half + 1) * hb * W].rearrange(
                        "p (b j) -> p b j", j=W
                    ),
                    in_=x_d.rearrange("(b p) j -> p b j", p=P)[
                        :, half * hb : (half + 1) * hb, :
                    ],
                )

            # ---- Y rows: y_loc[f, j] = (idx[j] - s[f]) * (-0.5*s[f]) ----
            sm_col = cpool.tile([F_LOC, 1], fp32)
            nc.vector.tensor_scalar_mul(sm_col[:, :], s_col[:, :], -0.5)
            y_loc = cpool.tile([F_LOC, W], fp32)
            nc.vector.tensor_scalar(
                y_loc[:, :],
                idx8_sb[:, :],
                s_col[:, 0:1],
                sm_col[:, 0:1],
                AOP.subtract,
                AOP.mult,
            )

            # ---- start the Y flatten roundtrip (needs y_loc) ----
            # partition_broadcast needs its source on partition 0, so first
            # flatten y_loc's 8 partition-rows into one row via DRAM scratch.
            ysc = dpool.tile([F_LOC, W], fp32)
            nc.sync.dma_start(out=ysc[:, :], in_=y_loc[:, :])
            y_rows = cpool.tile([1, F_LOC * W], fp32)
            nc.sync.dma_start(
                out=y_rows[:, :], in_=ysc[:, :].flatten().unsqueeze(0)
            )

            # ---- X columns (runs while the Y roundtrip is in flight):
            # xc[p, f*NB+b] = (c[f] - idx[b*128+p]) * 0.5*c[f] ----
            cB = cpool.tile([P, F_LOC], fp32)
            nc.gpsimd.partition_broadcast(cB[:, :], c_row[:, :])
            t1 = cpool.tile([P, F_LOC * NB], fp32)
            nc.vector.tensor_tensor(
                t1[:, :].rearrange("p (f b) -> p f b", b=NB),
                cB[:, :].unsqueeze(2).broadcast_to([P, F_LOC, NB]),
                idxcol_sb[:, :].unsqueeze(1).broadcast_to([P, F_LOC, NB]),
                AOP.subtract,
            )
            ch = cpool.tile([P, F_LOC], fp32)
            nc.vector.tensor_scalar_mul(ch[:, :], cB[:, :], 0.5)
            xc = cpool.tile([P, F_LOC * NB], fp32)
            nc.vector.tensor_tensor(
                xc[:, :].rearrange("p (f b) -> p f b", b=NB),
                t1[:, :].rearrange("p (f b) -> p f b", b=NB),
                ch[:, :].unsqueeze(2).broadcast_to([P, F_LOC, NB]),
                AOP.mult,
            )

            # ---- YB[f] = Y[f, :] broadcast to all 128 partitions,
            # per filter so the main loop can start on f=0 while later
            # filters are still broadcasting ----
            yb = cpool.tile([P, F_LOC * W], fp32)
            for f in range(F_LOC):
                nc.gpsimd.partition_broadcast(
                    yb[:, f * W : (f + 1) * W], y_rows[:, f * W : (f + 1) * W]
                )

            # ---- main loop: one fused op per [128, 1024] output tile ----
            out_r = out_d.rearrange("f (g p) j -> f p g j", p=P)
            n_dma = 0
            for f in range(F_LOC):
                for h in range(NB // GH):
                    big = opool.tile([P, GH * W], fp32, tag="big")
                    for k in range(GH):
                        b = h * GH + k
                        q = f * NB + b
                        nc.vector.scalar_tensor_tensor(
                            big[:, k * W : (k + 1) * W],
                            x_sb[:, b * W : (b + 1) * W],
                            xc[:, q : q + 1],
                            yb[:, f * W : (f + 1) * W],
                            AOP.add,
                            AOP.add,
                        )
                    dma_eng = nc.sync if n_dma % 2 == 0 else nc.scalar
                    n_dma += 1
                    dma_eng.dma_start(
                        out=out_r[f, :, h * GH : (h + 1) * GH, :],
                        in_=big[:, :].rearrange("p (g j) -> p g j", j=W),
                    )
    nc.compile()
    return nc


def _get_module():
    if "nc" not in _module_cache:
        _module_cache["nc"] = _build_module()
    return _module_cache["nc"]


def _host_inputs(x, filters):
    x = np.ascontiguousarray(x, dtype=np.float32)
    filters = np.asarray(filters, dtype=np.float32).reshape(F_TOTAL)
    c = np.cos(filters)
    s = np.sin(filters)
    denom = np.float32(W) * np.sqrt(np.float32(2.0))
    idx = (np.arange(W, dtype=np.float32) - np.float32(W / 2 - 0.5)) / denom
    idx8 = np.ascontiguousarray(np.broadcast_to(idx, (F_LOC, W)))
    idxcol = np.ascontiguousarray(idx.reshape(NB, P).T)  # [128, 8]
    in_maps = []
    for core in range(N_CORES):
        sl = slice(core * F_LOC, (core + 1) * F_LOC)
        in_maps.append(
            {
                "x": x,
                "cs": np.ascontiguousarray(np.stack([c[sl], s[sl]])),
                "idx8": idx8,
                "idxcol": idxcol,
            }
        )
    return in_maps


def kernel(x, filters):
    global LAST_RESULT
    import concourse.bass_utils as bass_utils

    nc = _get_module()
    in_maps = _host_inputs(x, filters)
    res = bass_utils.run_bass_kernel_spmd(
        nc,
        in_maps,
        core_ids=list(range(N_CORES)),
        trace=TRACE,
        stitch_traces=False,
    )
    LAST_RESULT = res
    return np.concatenate([r["out"] for r in res.results], axis=0)


# revision 13
# speedup vs baseline: 1.1067x; 1.1067x over previous
"""Trainium2 Bass kernel for nn_DirectionalDiagram.

out[f, i, j] = X[f, i] + Y[f, j] + x[i, j]        f in [64], i,j in [1024]
  X[f, i] = (cos(t_f) - idx[i]) * 0.5 * cos(t_f)
  Y[f, j] = (sin(t_f) - idx[j]) * 0.5 * sin(t_f)
  idx[i]  = (i - 511.5) / (1024 * sqrt(2))

Sharding: the filter axis is split across the 8 NeuronCores (8 filters per
core); x is replicated.  Per core the kernel is output-bandwidth bound
(32 MiB of f32 writes); the whole computation is a single fused DVE
scalar_tensor_tensor per [128, 1024] output tile:
    out_tile = (x_tile + X_col[f,b]) + YB[f]
where X_col is a per-partition scalar column and YB[f] is Y[f, :]
broadcast across partitions.  Partition broadcasts are built with K=1
TensorE matmuls (ones[1,128].T @ row) into PSUM + ScalarE copies to SBUF,
keeping the DVE (and its SBUF ports) dedicated to the fused main loop.
"""

import numpy as np

W = 1024          # image side
P = 128           # SBUF partitions
NB = W // P       # 8 row-blocks
F_TOTAL = 64
N_CORES = 8
F_LOC = F_TOTAL // N_CORES   # 8 filters per core
GH = 4            # row-blocks per output DMA (2 MiB per dma_start)
HN = 512          # matmul free-dim chunk (one PSUM bank)
# aux input column layout: idx8 | s | -0.5*s | c row | ones row | one-hot sel
AUX_S = 1024
AUX_SM = 1025
AUX_C = 1026
AUX_ONES = 1034
AUX_SEL = 1162
AUXW = AUX_SEL + 8 * 128

TRACE = False     # set by test harness to capture an NTFF profile
LAST_RESULT = None

_module_cache = {}


def _build_module():
    import concourse.bacc as bacc
    import concourse.mybir as mybir
    from concourse import tile

    fp32 = mybir.dt.float32
    AOP = mybir.AluOpType

    nc = bacc.Bacc("TRN2", target_bir_lowering=False, debug=False)
    x_d = nc.dram_tensor("x", [P, NB * W], fp32, kind="ExternalInput").ap()
    aux_d = nc.dram_tensor("aux", [F_LOC, AUXW], fp32, kind="ExternalInput").ap()
    idxcol_d = nc.dram_tensor("idxcol", [P, NB], fp32, kind="ExternalInput").ap()
    out_d = nc.dram_tensor("out", [F_LOC, W, W], fp32, kind="ExternalOutput").ap()

    with tile.TileContext(nc) as tc:
        with (
            tc.tile_pool(name="const", bufs=1) as cpool,
            tc.tile_pool(name="outp", bufs=4) as opool,
            tc.tile_pool(name="pcb", bufs=1, space="PSUM") as pcb,
            tc.tile_pool(name="pyb", bufs=4, space="PSUM") as pyb,
        ):
            # ---- tiny inputs on the scalar HWDGE ring ----
            aux_sb = cpool.tile([F_LOC, AUXW], fp32)
            nc.scalar.dma_start(out=aux_sb[:, :], in_=aux_d[:, :])
            idxcol_sb = cpool.tile([P, NB], fp32)
            nc.scalar.dma_start(out=idxcol_sb[:, :], in_=idxcol_d[:, :])

            # ---- x (host-pretransposed to [128, 8*1024]) in quarters on the
            # sync ring, ahead of the output DMAs ----
            x_sb = cpool.tile([P, NB * W], fp32)
            QW = NB * W // 4
            for q in range(4):
                nc.sync.dma_start(
                    out=x_sb[:, q * QW : (q + 1) * QW],
                    in_=x_d[:, q * QW : (q + 1) * QW],
                )

            # ---- Y rows: y_loc[f, j] = (idx[j] - s[f]) * (-0.5*s[f]) ----
            y_loc = cpool.tile([F_LOC, W], fp32)
            nc.vector.tensor_scalar(
                y_loc[:, :],
                aux_sb[:, 0:W],
                aux_sb[:, AUX_S : AUX_S + 1],
                aux_sb[:, AUX_SM : AUX_SM + 1],
                AOP.subtract,
                AOP.mult,
            )

            # ---- cB[p, f] = c[f] via ones.T @ c_row, PSUM -> SBUF on ACT ----
            cbp = pcb.tile([P, F_LOC], fp32)
            nc.tensor.matmul(
                cbp[:, :],
                aux_sb[0:1, AUX_ONES : AUX_ONES + P],
                aux_sb[0:1, AUX_C : AUX_C + F_LOC],
                start=True, stop=True,
            )
            cB = cpool.tile([P, F_LOC], fp32)
            nc.scalar.copy(cB[:, :], cbp[:, :])

            # ---- X columns: xc[p, f*NB+b] = (c[f] - idx[b*128+p]) * 0.5*c[f]
            t1 = cpool.tile([P, F_LOC * NB], fp32)
            nc.vector.tensor_tensor(
                t1[:, :].rearrange("p (f b) -> p f b", b=NB),
                cB[:, :].unsqueeze(2).broadcast_to([P, F_LOC, NB]),
                idxcol_sb[:, :].unsqueeze(1).broadcast_to([P, F_LOC, NB]),
                AOP.subtract,
            )
            ch = cpool.tile([P, F_LOC], fp32)
            nc.vector.tensor_scalar_mul(ch[:, :], cB[:, :], 0.5)
            xc = cpool.tile([P, F_LOC * NB], fp32)
            nc.vector.tensor_tensor(
                xc[:, :].rearrange("p (f b) -> p f b", b=NB),
                t1[:, :].rearrange("p (f b) -> p f b", b=NB),
                ch[:, :].unsqueeze(2).broadcast_to([P, F_LOC, NB]),
                AOP.mult,
            )

            # ---- YB[f] = Y[f, :] broadcast to 128 partitions via one-hot
            # selector matmuls ----
            yb = cpool.tile([P, F_LOC * W], fp32)
            for f in range(F_LOC):
                for hf in range(W // HN):
                    ps = pyb.tile([P, HN], fp32, tag="ybp")
                    nc.tensor.matmul(
                        ps[:, :],
                        aux_sb[:, AUX_SEL + f * P : AUX_SEL + (f + 1) * P],
                        y_loc[:, hf * HN : (hf + 1) * HN],
                        start=True, stop=True,
                    )
                    nc.scalar.copy(
                        yb[:, f * W + hf * HN : f * W + (hf + 1) * HN], ps[:, :]
                    )

            # ---- main loop: one fused op per [128, 1024] output tile ----
            out_r = out_d.rearrange("f (g p) j -> f p g j", p=P)
            for f in range(F_LOC):
                gh = 1 if f == 0 else GH
                for h in range(NB // gh):
                    big = opool.tile([P, GH * W], fp32, tag="big")
                    for k in range(gh):
                        b = h * gh + k
                        q = f * NB + b
                        nc.vector.scalar_tensor_tensor(
                            big[:, k * W : (k + 1) * W],
                            x_sb[:, b * W : (b + 1) * W],
                            xc[:, q : q + 1],
                            yb[:, f * W : (f + 1) * W],
                            AOP.add,
                            AOP.add,
                        )
                    nc.sync.dma_start(
                        out=out_r[f, :, h * gh : (h + 1) * gh, :],
                        in_=big[:, : gh * W].rearrange("p (g j) -> p g j", j=W),
                    )
    nc.compile()
    return nc


def _get_module():
    if "nc" not in _module_cache:
        _module_cache["nc"] = _build_module()
    return _module_cache["nc"]


def _host_inputs(x, filters):
    x = np.asarray(x, dtype=np.float32)
    filters = np.asarray(filters, dtype=np.float32).reshape(F_TOTAL)
    # pre-transpose x to the SBUF layout [128, 8*1024] (block b at cols b*W)
    xr = np.ascontiguousarray(
        x.reshape(NB, P, W).transpose(1, 0, 2).reshape(P, NB * W)
    )
    c = np.cos(filters)
    s = np.sin(filters)
    denom = np.float32(W) * np.sqrt(np.float32(2.0))
    idx = (np.arange(W, dtype=np.float32) - np.float32(W / 2 - 0.5)) / denom
    idxcol = np.ascontiguousarray(idx.reshape(NB, P).T)  # [128, 8]
    in_maps = []
    for core in range(N_CORES):
        sl = slice(core * F_LOC, (core + 1) * F_LOC)
        aux = np.zeros((F_LOC, AUXW), dtype=np.float32)
        aux[:, 0:W] = idx[None, :]
        aux[:, AUX_S] = s[sl]
        aux[:, AUX_SM] = np.float32(-0.5) * s[sl]
        aux[0, AUX_C : AUX_C + F_LOC] = c[sl]
        aux[0, AUX_ONES : AUX_ONES + P] = np.float32(1.0)
        sel = np.eye(F_LOC, dtype=np.float32).repeat(P, axis=1)
        aux[:, AUX_SEL:] = np.kron(
            np.eye(F_LOC, dtype=np.float32), np.ones((1, P), dtype=np.float32)
        )
        in_maps.append({"x": xr, "aux": aux, "idxcol": idxcol})
    return in_maps


def kernel(x, filters):
    global LAST_RESULT
    import concourse.bass_utils as bass_utils

    nc = _get_module()
    in_maps = _host_inputs(x, filters)
    res = bass_utils.run_bass_kernel_spmd(
        nc,
        in_maps,
        core_ids=list(range(N_CORES)),
        trace=TRACE,
        stitch_traces=False,
    )
    LAST_RESULT = res
    return np.concatenate([r["out"] for r in res.results], axis=0)
